# revision 18
# baseline (speedup 1.0000x reference)
"""Trainium2 Bass kernel for the LIF forward + e-prop eligibility-trace scan.

Math (per batch row, per step):
    v_t = a*v_{t-1} + x_t.w          (a = 0.995)
    z_t = 1[v_t > 2]
    eps = x_t - v_t w
    g  += v_t*eps + (eps.w) p_{t-1}  ;  p_t = a*p_{t-1} + x_t

Using eps.w = s_t - c*v_t (s = x@w, c = w.w) everything reduces to
    g = sum_t (v_t + q_t) x_t - (sum_t v_t^2) w
with q a backward exponential filter of a_t = s_t - c*v_t.  Computed in a
single streaming pass over x with chunked time: within a chunk the backward
filter is local; cross-chunk history flows through p (also an exponentially
weighted sum of x, accumulated with constant weights on the tensor engine).

Sharding: data-parallel over batch, 16 rows per core, 8 cores; w replicated;
outputs concatenated (g is per-row here, so no cross-core reduction).
"""

import os
from contextlib import ExitStack

import numpy as np

import concourse.bass as bass
import concourse.tile as tile
from concourse import bacc, mybir

ALPHA = 1.0 - 0.05 / 10.0  # 0.995
V_TH = 2.0

F32 = mybir.dt.float32
AOP = mybir.AluOpType
AX = mybir.AxisListType

N_CORES = 8
B_FULL, T_FULL, N_FULL = 128, 2000, 512


def _rev_free(ap):
    """Reversed view along the (single) free dim of a 2-D AP."""
    (p_step, p_cnt), (f_step, f_cnt) = list(ap.ap)
    assert f_step == 1
    return bass.AP(
        tensor=ap.tensor,
        offset=ap.offset + (f_cnt - 1),
        ap=[[p_step, p_cnt], [-1, f_cnt]],
    )


def build_nc(B=16, T=2000, N=512, TC=256):
    """Build the single-core Bass kernel (same program for all 8 cores)."""
    assert T % 8 == 0 and TC % 8 == 0
    chunks = []
    t0 = 0
    while t0 < T:
        L = min(TC, T - t0)
        assert L % 8 == 0
        chunks.append((t0, L))
        t0 += L
    n_chunks = len(chunks)
    max_tiles = TC // 8

    nc = bacc.Bacc("TRN2", target_bir_lowering=False, debug=False)
    x = nc.dram_tensor("x", [B, T, N], F32, kind="ExternalInput")
    w128 = nc.dram_tensor("w128", [128, N], F32, kind="ExternalInput")
    apow = nc.dram_tensor("apow", [B, TC], F32, kind="ExternalInput")
    wspc = nc.dram_tensor("wspc", [128, 48 * max_tiles], F32, kind="ExternalInput")
    eye32 = nc.dram_tensor("eye32", [48, 16], F32, kind="ExternalInput")
    cneg = nc.dram_tensor("cneg", [B, 1], F32, kind="ExternalInput")
    v_out = nc.dram_tensor("v_out", [B, T], F32, kind="ExternalOutput")
    z_out = nc.dram_tensor("z_out", [B, T], F32, kind="ExternalOutput")
    g_out = nc.dram_tensor("g_out", [B, N], F32, kind="ExternalOutput")

    x_4d = x[:, :, :].rearrange("b (tt j) n -> tt b j n", j=8)  # [T/8, B, 8, N]
    scr = [nc.dram_tensor(f"scr{i}", [128, max_tiles], F32) for i in range(2)]
    rdr = [nc.dram_tensor(f"rdr{i}", [B, TC], F32) for i in range(2)]

    with tile.TileContext(nc) as tc, ExitStack() as ctx:
        consts = ctx.enter_context(tc.tile_pool(name="consts", bufs=1))
        xpool = ctx.enter_context(tc.tile_pool(name="xp", bufs=2 * max_tiles))
        junkp = ctx.enter_context(tc.tile_pool(name="junk", bufs=2))
        smallp = ctx.enter_context(tc.tile_pool(name="small", bufs=3))
        gpool = ctx.enter_context(tc.tile_pool(name="gp", bufs=3))
        ppool = ctx.enter_context(tc.tile_pool(name="pp", bufs=2))
        psump = ctx.enter_context(
            tc.tile_pool(name="psum", bufs=2, space=bass.MemorySpace.PSUM)
        )

        w128_t = consts.tile([128, N], F32)
        nc.gpsimd.dma_start(out=w128_t[:], in_=w128[:, :])
        apow_t = consts.tile([B, TC], F32)
        nc.gpsimd.dma_start(out=apow_t[:], in_=apow[:, :])
        cneg_t = consts.tile([B, 1], F32)
        nc.gpsimd.dma_start(out=cneg_t[:], in_=cneg[:, :])
        eye_t = consts.tile([48, 16], F32)
        nc.gpsimd.dma_start(out=eye_t[:], in_=eye32[:, :])
        wsp_t = []
        for i in range(2):
            wt = consts.tile([128, 48 * max_tiles], F32, tag=f"wsp{i}")
            nc.gpsimd.dma_start(out=wt[:], in_=wspc[:, :])
            wsp_t.append(wt)
        alpha_t = consts.tile([B, TC], F32)
        nc.vector.memset(alpha_t[:], ALPHA)
        v_all = consts.tile([B, T], F32)
        z_all = consts.tile([B, T], F32)
        sv2 = consts.tile([B, n_chunks], F32)

        g_prev = None
        p_prev = None
        for ci, (t0, L) in enumerate(chunks):
            nt = L // 8
            last = ci == n_chunks - 1
            wsp = wsp_t[ci % 2]

            # --- stream x tiles; fused multiply-by-w + free-dim reduce -> s
            stg = smallp.tile([128, max_tiles], F32, tag="stg")
            xts = []
            for c in range(nt):
                xt = xpool.tile([128, N], F32, tag="xt")
                nc.sync.dma_start(out=xt[:], in_=x_4d[t0 // 8 + c])
                junk = junkp.tile([128, N], F32, tag="junk")
                nc.vector.scalar_tensor_tensor(
                    out=junk[:],
                    in0=xt[:],
                    scalar=1.0,
                    in1=w128_t[:],
                    op0=AOP.bypass,
                    op1=AOP.mult,
                    accum_out=stg[:, c : c + 1],
                )
                xts.append(xt)

            # --- gather s from (b*8+j, c) staging into [b, t] layout.
            # Partition-reordering SBUF APs confuse Tile's dependency
            # tracking, so bounce through DRAM: SBUF-side APs stay plain and
            # the reordering lives in the DRAM-side APs.
            s_ch = smallp.tile([B, TC], F32, tag="sch")
            sc = scr[ci % 2]
            nc.sync.dma_start(out=sc[:, 0:nt], in_=stg[:, 0:nt])
            scr_j = sc[:, 0:nt].rearrange("(b j) c -> j b c", j=8)
            sch_j = s_ch[:, 0:L].rearrange("b (c j) -> j b c", j=8)
            for j in range(8):
                nc.gpsimd.dma_start(out=sch_j[j], in_=scr_j[j])

            # --- v scan (chained via carry), a, local backward filter u, r
            v_dst = v_all[:, t0 : t0 + L]
            init = 0.0 if ci == 0 else v_all[:, t0 - 1 : t0]
            nc.vector.tensor_tensor_scan(
                out=v_dst, data0=alpha_t[:, 0:L], data1=s_ch[:, 0:L],
                initial=init, op0=AOP.mult, op1=AOP.add,
            )
            a_ch = smallp.tile([B, TC], F32, tag="ach")
            nc.vector.scalar_tensor_tensor(
                out=a_ch[:, 0:L], in0=v_dst, scalar=cneg_t[:, 0:1],
                in1=s_ch[:, 0:L], op0=AOP.mult, op1=AOP.add,
            )
            u_m = smallp.tile([B, TC], F32, tag="um")
            nc.vector.tensor_tensor_scan(
                out=u_m[:, 0:L], data0=alpha_t[:, 0:L],
                data1=_rev_free(a_ch[:, 0:L]),
                initial=0.0, op0=AOP.mult, op1=AOP.add,
            )
            # r_t = v_t + u_{t+1};   u_m[m] holds u_{L-1-m}
            r_ch = smallp.tile([B, TC], F32, tag="rch")
            nc.vector.tensor_add(
                r_ch[:, 0 : L - 1],
                v_all[:, t0 : t0 + L - 1],
                _rev_free(u_m[:, 0 : L - 1]),
            )
            nc.vector.tensor_copy(
                r_ch[:, L - 1 : L], v_all[:, t0 + L - 1 : t0 + L]
            )

            # --- per-chunk reductions: A_c = sum a_i a^i ; sum v^2 (negated)
            A_col = smallp.tile([B, 1], F32, tag="acol")
            junk2 = smallp.tile([B, TC], F32, tag="jk2")
            nc.vector.scalar_tensor_tensor(
                out=junk2[:, 0:L], in0=a_ch[:, 0:L], scalar=1.0,
                in1=apow_t[:, 0:L], op0=AOP.bypass, op1=AOP.mult,
                accum_out=A_col[:],
            )
            junk3 = smallp.tile([B, TC], F32, tag="jk3")
            nc.vector.scalar_tensor_tensor(
                out=junk3[:, 0:L], in0=v_dst, scalar=-1.0,
                in1=v_dst, op0=AOP.mult, op1=AOP.mult,
                accum_out=sv2[:, ci : ci + 1],
            )

            # --- write r into the per-tile weight columns of wsp via a
            # DRAM bounce (16 per-b DMAs, plain SBUF-side APs).
            rd = rdr[ci % 2]
            nc.sync.dma_start(out=rd[:, 0:L], in_=r_ch[:, 0:L])
            rdr_b = rd[:, 0:L].rearrange("b (c j) -> b j c", j=8)
            wsp_b = wsp[:, 0 : 48 * nt].rearrange(
                "(b j) (c k) -> b k j c", j=8, k=48
            )
            for b in range(16):
                nc.gpsimd.dma_start(out=wsp_b[b, b], in_=rdr_b[b])

            # --- weighted sums over the chunk's x tiles (PE); the A_c*p_in
            # cross-chunk term rides the same accumulation as a diag matmul.
            psum = psump.tile([48, N], F32, tag="ps")
            for c in range(nt):
                nc.tensor.matmul(
                    psum[:],
                    wsp[:, c * 48 : (c + 1) * 48],
                    xts[c][:],
                    start=(c == 0),
                    stop=(c == nt - 1),
                )
                if c == 0 and ci > 0:
                    # diagA (partitions 16:32) built from A_col
                    A2 = smallp.tile([48, 1], F32, tag="a2")
                    nc.sync.dma_start(out=A2[32:48, :], in_=A_col[:])
                    diagA = smallp.tile([48, 16], F32, tag="diag")
                    nc.vector.tensor_scalar(
                        out=diagA[32:48, :], in0=eye_t[32:48, :],
                        scalar1=A2[32:48, 0:1], scalar2=None, op0=AOP.mult,
                    )
                    nc.tensor.matmul(
                        psum[0:16, :], diagA[32:48, :], p_prev[32:48, :],
                        start=False, stop=False, skip_group_check=True,
                    )

            # --- g / p chunk updates (p lives on partitions 16:32)
            if ci == 0:
                g_new = gpool.tile([B, N], F32, tag="g")
                nc.vector.tensor_copy(g_new[:], psum[0:16, :])
                g_prev = g_new
            else:
                g_new = gpool.tile([B, N], F32, tag="g")
                nc.vector.tensor_add(g_new[:], g_prev[:], psum[0:16, :])
                g_prev = g_new
            if not last:
                p_stage = ppool.tile([48, N], F32, tag="pstage")
                nc.scalar.activation(
                    out=p_stage[32:48, :], in_=psum[32:48, :],
                    func=mybir.ActivationFunctionType.Copy,
                )
                if ci == 0:
                    p_prev = p_stage
                else:
                    p_new = ppool.tile([48, N], F32, tag="p")
                    nc.vector.scalar_tensor_tensor(
                        out=p_new[32:48, :], in0=p_prev[32:48, :],
                        scalar=float(ALPHA**L),
                        in1=p_stage[32:48, :], op0=AOP.mult, op1=AOP.add,
                    )
                    p_prev = p_new

        # --- epilogue: g -= (sum v^2) w  (sv2 holds negated sums); z; DMA out
        sv2tot = smallp.tile([B, 1], F32, tag="sv2t")
        nc.vector.reduce_sum(sv2tot[:], sv2[:], axis=AX.X)
        g_fin = gpool.tile([B, N], F32, tag="g")
        nc.vector.scalar_tensor_tensor(
            out=g_fin[:], in0=w128_t[0:16, :], scalar=sv2tot[:, 0:1],
            in1=g_prev[:], op0=AOP.mult, op1=AOP.add,
        )
        nc.vector.tensor_scalar(
            out=z_all[:], in0=v_all[:], scalar1=V_TH, scalar2=None,
            op0=AOP.is_gt,
        )
        nc.sync.dma_start(out=v_out[:, :], in_=v_all[:])
        nc.sync.dma_start(out=z_out[:, :], in_=z_all[:])
        nc.sync.dma_start(out=g_out[:, :], in_=g_fin[:])

    nc.compile()
    return nc


def make_consts(w, B=16, TC=256):
    """Host-side constant tensors shared by all cores."""
    w = np.asarray(w, dtype=np.float32)
    N = w.shape[0]
    max_tiles = TC // 8
    w128 = np.tile(w[None, :], (128, 1)).astype(np.float32)
    apow = np.tile(
        (np.float32(ALPHA) ** np.arange(TC, dtype=np.float32))[None, :], (B, 1)
    ).astype(np.float32)
    wspc = np.zeros((128, 48 * max_tiles), dtype=np.float32)
    for c in range(max_tiles):
        for b in range(16):
            for j in range(8):
                wspc[b * 8 + j, c * 48 + 32 + b] = np.float32(ALPHA) ** (
                    TC - 1 - (c * 8 + j)
                )
    c_val = np.float32(np.dot(w.astype(np.float32), w.astype(np.float32)))
    cneg = np.full((B, 1), -c_val, dtype=np.float32)
    eye32 = np.zeros((48, 16), dtype=np.float32)
    eye32[32:48, :] = np.eye(16, dtype=np.float32)
    return {
        "w128": w128,
        "apow": apow,
        "wspc": wspc,
        "eye32": eye32,
        "cneg": cneg,
    }


_NC_CACHE = {}


def _get_nc(key=(16, 2000, 512, 256)):
    if key not in _NC_CACHE:
        _NC_CACHE[key] = build_nc(*key)
    return _NC_CACHE[key]


def kernel(x, w):
    from concourse.bass_utils import run_bass_kernel_spmd

    x = np.ascontiguousarray(np.asarray(x, dtype=np.float32))
    w = np.asarray(w, dtype=np.float32)
    assert x.shape == (B_FULL, T_FULL, N_FULL), x.shape
    Bc = B_FULL // N_CORES

    nc = _get_nc()
    consts = make_consts(w)
    in_maps = []
    for core in range(N_CORES):
        m = dict(consts)
        m["x"] = np.ascontiguousarray(x[core * Bc : (core + 1) * Bc])
        in_maps.append(m)

    trace = bool(int(os.environ.get("KERNEL_TRACE", "0")))
    res = run_bass_kernel_spmd(
        nc, in_maps, core_ids=list(range(N_CORES)), trace=trace
    )
    if trace and res.exec_time_ns is not None:
        print(f"HW exec time: {res.exec_time_ns} ns")
        kernel.last_exec_time_ns = res.exec_time_ns
        kernel.last_trace = res.instructions_and_trace
    v = np.concatenate([r["v_out"] for r in res.results], axis=0)
    z = np.concatenate([r["z_out"] for r in res.results], axis=0)
    g = np.concatenate([r["g_out"] for r in res.results], axis=0)
    return v, z, g


if __name__ == "__main__":
    nc = build_nc()
    print("built ok")
